# revision 1
# baseline (speedup 1.0000x reference)
"""ColBERT MaxSim retrieval kernel for 8 Trainium2 NeuronCores.

Problem (full shapes):
  query_hidden [64,32,768], doc_hidden [256,180,768], query_mask [64,32],
  doc_punct_mask [256,180], W1 [768,768], b1 [768], W2 [768,128], b2 [128]
  out [64, 256]:
    qe = l2norm(relu(qh@W1+b1)@W2+b2 * qm)        # [64,32,128]
    de = l2norm(relu(dh@W1+b1)@W2+b2 * dm)        # [256,180,128]
    s  = einsum('qih,djh->qidj', qe, de) * dm
    out = s.max(-1).sum(1) / qm.sum(-1, keepdims=True)

Sharding: docs split across the 8 cores (32 docs each); queries are
replicated.  Embarrassingly parallel - no collectives.

Host-side mask compaction (exact, not approximate):
  - masked query tokens contribute exactly 0 (their weight in the final
    per-query sum is qm/qsum = 0), so only unmasked query tokens are
    shipped, padded up to a multiple of 512 with zero rows whose weight
    rows in the indicator matrix are 0.
  - masked doc tokens only contribute the "0" baseline to the per-doc max
    (reference multiplies scores by dm before the max).  Each doc keeps its
    unmasked tokens plus >=1 zero-padded slot (rd=0 -> score exactly 0),
    preserving that baseline.  Docs are re-laid at a fixed stride LDP
    (128, or 192 in the cosmically-unlikely case some doc has >=128
    unmasked tokens; ld=180 bounds it).

Math rearrangement (exactly equivalent up to fp rounding):
  e_masked_normed = e_raw * (mask / max(||e_raw||, eps))  per token
  -> scale de^T columns by rd = dm/max(||e_raw||,eps); the query-side
     factor rq >= 0 commutes with the max over doc tokens, so it is
     applied to the per-(qtok,doc) maxima; query mask and the 1/qm.sum()
     normalizer live in a host-built block-indicator matrix used as the
     lhsT of the final reduction matmul.

All embeddings are produced directly in transposed [E/H on partitions,
tokens free] layout so every matmul contraction lands on the partition
dim with zero on-chip transposes (activations are transposed host-side).
fp32r (full-rate fp32 PE path) is used for all large matmuls.
"""

import os
import sys

import numpy as np

for _p in ("/opt/trn_rl_repo",):
    if _p not in sys.path and os.path.isdir(_p):
        sys.path.insert(0, _p)

import concourse.bass as bass
import concourse.mybir as mybir
import concourse.tile as tile
from concourse.bass_utils import run_bass_kernel_spmd

F32 = mybir.dt.float32
F32R = mybir.dt.float32r

# problem dims
NQ, LQ, ND, LD, H, E = 64, 32, 256, 180, 768, 128
NCORES = 8
QT = NQ * LQ                 # 2048 query tokens total (pre-compaction)
NDC = ND // NCORES           # 32 docs per core
KC = H // 128                # 6 contraction chunks
TW = 512                     # token tile width (queries and docs)
EPS = 1e-12

_CACHE = {}


def _build_module(qtp, ldp, split_waits=True, repeats=1):
    """qtp: padded compacted query-token count (multiple of 512).
    ldp: per-doc token stride after compaction (2*ldp <= 512)."""
    dtp = NDC * ldp              # doc tokens per core
    nc = bass.Bass("TRN2", target_bir_lowering=False, debug=False,
                   num_devices=NCORES)

    ntd = dtp // TW
    dh = nc.dram_tensor("dht", [H, dtp], F32R, kind="ExternalInput").ap()
    qh = nc.dram_tensor("qht", [H, qtp], F32R, kind="ExternalInput").ap()
    w1 = nc.dram_tensor("w1", [H, H], F32R, kind="ExternalInput").ap()
    w2 = nc.dram_tensor("w2", [H, E], F32R, kind="ExternalInput").ap()
    b1 = nc.dram_tensor("b1c", [128, KC], F32, kind="ExternalInput").ap()
    b2 = nc.dram_tensor("b2c", [128, 1], F32, kind="ExternalInput").ap()
    dmr = nc.dram_tensor("dmr", [ntd, TW], F32, kind="ExternalInput").ap()
    id4 = nc.dram_tensor("id4", [4, 4], F32, kind="ExternalInput").ap()
    wind = nc.dram_tensor("wind", [qtp, NQ], F32, kind="ExternalInput").ap()
    out = nc.dram_tensor("out", [NQ, NDC], F32, kind="ExternalOutput").ap()

    with tile.TileContext(nc) as tc:
        for _ in range(repeats):
            _emit(tc, nc, qtp, ldp, dh, qh, w1, w2, b1, b2, dmr, wind, id4,
                  out)
    if split_waits:
        _split_multi_waits(nc)
    return nc


def _split_multi_waits(nc, max_waits=1):
    """This walrus build rejects instructions carrying more than one sync
    wait (e.g. the S3_LW stage of fused 4-byte matmuls, Drain). Hoist extra
    waits into standalone same-engine InstEventSemaphore instructions placed
    immediately before the offender - semantics are identical since each
    engine executes its stream in order."""
    n = 0
    for f in nc.m.functions:
        for bb in f.blocks:
            new = []
            for ins in bb.instructions:
                si = ins.sync_info
                waits = list(si.on_wait) if si is not None and si.on_wait else []
                if len(waits) > max_waits:
                    for sw in waits[:-max_waits]:
                        n += 1
                        new.append(mybir.InstEventSemaphore(
                            name=f"WS-{n}", engine=ins.engine, ins=[], outs=[],
                            sync_info=mybir.SyncInfo(on_wait=[sw], on_update=[])))
                    ins.sync_info = mybir.SyncInfo(
                        on_wait=waits[-max_waits:],
                        on_update=list(si.on_update) if si.on_update else [])
                new.append(ins)
            bb.instructions = new


def _emit(tc, nc, qtp, ldp, dh, qh, w1, w2, b1, b2, dmr, wind, id4, out):
    from contextlib import ExitStack

    dtp = NDC * ldp
    ntd, ntq = dtp // TW, qtp // TW
    nqch = qtp // 128            # 128-token query chunks
    # score-tile width: as many whole docs as fit a 512-wide psum bank
    dpg = 512 // ldp             # docs per score tile (4 @ ldp=128)
    dg = dpg * ldp
    ndg = NDC // dpg
    selw = 2 * ntd - 1

    with ExitStack() as ctx:
        cp = ctx.enter_context(tc.tile_pool(name="consts", bufs=1))
        w1_sb = cp.tile([128, KC, H], F32R, tag="w1sb")
        w2_sb = cp.tile([128, KC, E], F32R, tag="w2sb")
        b1_sb = cp.tile([128, KC], F32, tag="b1sb")
        b2_sb = cp.tile([128, 1], F32, tag="b2sb")
        dmr_sb = cp.tile([ntd, TW], F32, tag="dmrsb")
        wind_sb = cp.tile([128, nqch, NQ], F32, tag="windsb")
        # selector: all zeros except column ntd-1 (all ones); slicing
        # sel[:, ntd-1-t : ntd-1-t+M] -> lhsT whose only non-zero column
        # is t, so the ones-reduction lands in psum row t.
        sel_sb = cp.tile([128, selw], F32R, tag="selsb")
        id_sb = cp.tile([4, 4], F32, tag="idsb")
        ones_row = cp.tile([1, 128], F32R, tag="onesrow")
        deT = cp.tile([128, dtp], F32R, tag="deT")
        qeT = cp.tile([128, qtp], F32R, tag="qeT")
        rq_all = cp.tile([128, nqch], F32, tag="rqall")
        rd_sb = cp.tile([ntd, TW], F32R, tag="rdsb")
        rd_row = cp.tile([1, dtp], F32R, tag="rdrow")
        mq_sb = cp.tile([ntq, TW], F32, tag="mqsb")
        md_sb = cp.tile([ntd, TW], F32, tag="mdsb")
        out_sb = cp.tile([NQ, NDC], F32, tag="outsb")

        nc.sync.dma_start(out=w1_sb[:], in_=w1.rearrange("(k p) h -> p k h", p=128))
        nc.sync.dma_start(out=w2_sb[:], in_=w2.rearrange("(k p) e -> p k e", p=128))
        nc.sync.dma_start(out=b1_sb[:], in_=b1)
        nc.sync.dma_start(out=b2_sb[:], in_=b2)
        nc.sync.dma_start(out=dmr_sb[:], in_=dmr)
        nc.sync.dma_start(out=wind_sb[:], in_=wind.rearrange("(g p) q -> p g q", p=128))
        nc.sync.dma_start(out=id_sb[:], in_=id4)
        # memset can't target f32r; build in f32 scratch and copy (the
        # tensor_copy converts, which satisfies the fp32r rounding rule)
        zsc = cp.tile([128, selw], F32, tag="zsc")
        nc.vector.memset(zsc[:], 0.0)
        nc.vector.memset(zsc[:, ntd - 1:ntd], 1.0)
        nc.vector.tensor_copy(sel_sb[:], zsc[:])
        osc = cp.tile([1, 128], F32, tag="osc")
        nc.vector.memset(osc[:], 1.0)
        nc.vector.tensor_copy(ones_row[:], osc[:])

        io_pool = ctx.enter_context(tc.tile_pool(name="io", bufs=5))
        h1_pool = ctx.enter_context(tc.tile_pool(name="h1", bufs=3))
        sq_pool = ctx.enter_context(tc.tile_pool(name="sq", bufs=2))

        def head_tile(src, t, nt, et_dst, psq_acc):
            """MLP head for one 512-token tile; writes e^T into et_dst
            ([128, TW] slice) and accumulates the per-token sum of squares
            into row t of psq_acc [nt, TW] via a selector matmul."""
            xt = io_pool.tile([128, KC, TW], F32R, tag="xt")
            nc.sync.dma_start(out=xt[:],
                              in_=src.rearrange("(k p) n -> p k n", p=128)
                              [:, :, t * TW:(t + 1) * TW])
            h1 = h1_pool.tile([128, KC, TW], F32R, tag="h1")
            for h in range(KC):
                ph = ph_pool.tile([128, TW], F32, tag="ph")
                for k in range(KC):
                    nc.tensor.matmul(
                        ph[:],
                        w1_sb[:, k, h * 128:(h + 1) * 128],
                        xt[:, k, :],
                        start=(k == 0), stop=(k == KC - 1))
                nc.scalar.activation(h1[:, h, :], ph[:],
                                     mybir.ActivationFunctionType.Relu,
                                     bias=b1_sb[:, h:h + 1])
            pe = pe_pool.tile([128, TW], F32, tag="pe")
            for h in range(KC):
                nc.tensor.matmul(pe[:], w2_sb[:, h, :], h1[:, h, :],
                                 start=(h == 0), stop=(h == KC - 1))
            nc.scalar.activation(et_dst, pe[:],
                                 mybir.ActivationFunctionType.Identity,
                                 bias=b2_sb[:, 0:1])
            sq = sq_pool.tile([128, TW], F32R, tag="sq")
            nc.gpsimd.tensor_mul(sq[:], et_dst, et_dst)
            nc.tensor.matmul(psq_acc, sel_sb[:, ntd - 1 - t:ntd - 1 - t + nt],
                             sq[:], start=(t == 0), stop=(t == nt - 1))

        with ExitStack() as pctx:
            ph_pool = pctx.enter_context(
                tc.tile_pool(name="ph", bufs=2, space="PSUM"))
            pe_pool = pctx.enter_context(
                tc.tile_pool(name="pex", bufs=2, space="PSUM"))
            psq_pool = pctx.enter_context(
                tc.tile_pool(name="psq", bufs=1, space="PSUM"))
            ptr_pool = pctx.enter_context(
                tc.tile_pool(name="ptr", bufs=1, space="PSUM"))

            # ---- docs ----
            psq_d = psq_pool.tile([ntd, TW], F32, tag="psqd")
            for t in range(ntd):
                head_tile(dh, t, ntd, deT[:, t * TW:(t + 1) * TW], psq_d[:])
            # rd = dm / max(sqrt(ssq), eps), in [ntd, TW] layout
            nc.vector.tensor_copy(md_sb[:], psq_d[:])
            nc.scalar.activation(md_sb[:], md_sb[:],
                                 mybir.ActivationFunctionType.Sqrt)
            nc.vector.tensor_scalar_max(md_sb[:], md_sb[:], EPS)
            with nc.allow_low_precision(reason="f32r has ample mantissa "
                                        "for unit-scale norm reciprocals"):
                nc.vector.reciprocal(rd_sb[:], md_sb[:])
                nc.vector.tensor_mul(rd_sb[:], rd_sb[:], dmr_sb[:])
            # re-lay [ntd, TW] -> one [1, dtp] row so every broadcast source
            # sits at partition base 0 (engines can't address base>0)
            nc.sync.dma_start(out=rd_row[:], in_=rd_sb[:])
            # scale deT columns by rd: K=1 ones-row matmul broadcasts each
            # [1, TW] slice across all 128 partitions via PSUM
            for t in range(ntd):
                bc = ph_pool.tile([128, TW], F32, tag="ph")
                nc.tensor.matmul(bc[:], ones_row[:],
                                 rd_row[:, t * TW:(t + 1) * TW],
                                 start=True, stop=True)
                sl = deT[:, t * TW:(t + 1) * TW]
                nc.vector.tensor_mul(sl, sl, bc[:])

            # ---- queries ----
            psq_q = psq_pool.tile([ntq, TW], F32, tag="psqq")
            for t in range(ntq):
                head_tile(qh, t, ntq, qeT[:, t * TW:(t + 1) * TW], psq_q[:])
            # rq = 1 / max(sqrt(ssq), eps)  (query mask folded into wind)
            nc.vector.tensor_copy(mq_sb[:], psq_q[:])
            nc.scalar.activation(mq_sb[:], mq_sb[:],
                                 mybir.ActivationFunctionType.Sqrt)
            nc.vector.tensor_scalar_max(mq_sb[:], mq_sb[:], EPS)
            nc.vector.reciprocal(mq_sb[:], mq_sb[:])
            # transpose [ntq, TW] rows -> rq_all [128, nqch] columns
            rq_v = rq_all[:].rearrange("p (t c) -> p t c", c=4)
            for c in range(4):
                ptr = ptr_pool.tile([128, 4], F32, tag="ptr")
                nc.tensor.transpose(ptr[:, :ntq],
                                    mq_sb[:, c * 128:(c + 1) * 128],
                                    id_sb[0:ntq, 0:ntq])
                nc.vector.tensor_copy(rq_v[:, :, c], ptr[:, :ntq])

        # ---- scores ----
        with ExitStack() as sctx:
            ps_pool = sctx.enter_context(
                tc.tile_pool(name="ps", bufs=6, space="PSUM"))
            po_pool = sctx.enter_context(
                tc.tile_pool(name="po", bufs=1, space="PSUM"))
            m_pool = sctx.enter_context(tc.tile_pool(name="m", bufs=nqch))

            m_tiles = []
            for g in range(nqch):
                qchunk = qeT[:, g * 128:(g + 1) * 128]
                mt = m_pool.tile([128, NDC], F32, tag="mt")
                for j in range(ndg):
                    ps = ps_pool.tile([128, dg], F32, tag="ps")
                    nc.tensor.matmul(ps[:], qchunk,
                                     deT[:, j * dg:(j + 1) * dg],
                                     start=True, stop=True)
                    nc.vector.tensor_reduce(
                        mt[:, j * dpg:(j + 1) * dpg],
                        ps[:].rearrange("p (d j) -> p d j", j=ldp),
                        axis=mybir.AxisListType.X, op=mybir.AluOpType.max)
                nc.vector.tensor_scalar_mul(mt[:], mt[:], rq_all[:, g:g + 1])
                m_tiles.append(mt)

            pout = po_pool.tile([NQ, NDC], F32, tag="pout")
            for g in range(nqch):
                nc.tensor.matmul(pout[:], wind_sb[:, g, :], m_tiles[g][:],
                                 start=(g == 0), stop=(g == nqch - 1))
            nc.vector.tensor_copy(out_sb[:], pout[:])
            nc.sync.dma_start(out=out, in_=out_sb[:])


def _get_module(qtp, ldp):
    key = ("nc", qtp, ldp)
    if key not in _CACHE:
        _CACHE[key] = _build_module(qtp, ldp)
    return _CACHE[key]


def _prep_inputs(query_hidden, doc_hidden, query_mask, doc_punct_mask,
                 W1, b1, W2, b2):
    """Host-side compaction + shard + layout prep.
    Returns (per-core input maps, qtp, ldp)."""
    f32 = np.float32
    qh2 = np.asarray(query_hidden, f32).reshape(QT, H)
    dh2 = np.asarray(doc_hidden, f32).reshape(ND * LD, H)
    qm = np.asarray(query_mask, f32).reshape(QT)
    dmf = np.asarray(doc_punct_mask, f32).reshape(ND, LD)
    w1 = np.ascontiguousarray(np.asarray(W1, f32))
    w2 = np.ascontiguousarray(np.asarray(W2, f32))
    b1c = np.ascontiguousarray(np.asarray(b1, f32).reshape(KC, 128).T)
    b2c = np.ascontiguousarray(np.asarray(b2, f32).reshape(E, 1))

    # ---- query compaction ----
    qidx = np.nonzero(qm > 0)[0]
    kq = len(qidx)
    qtp = max(TW, int(-(-kq // TW)) * TW)
    qh_c = np.zeros((qtp, H), f32)
    qh_c[:kq] = qh2[qidx]
    qht = np.ascontiguousarray(qh_c.T)
    qsum = qm.reshape(NQ, LQ).sum(axis=1)
    qsum = np.maximum(qsum, 1.0)
    wind = np.zeros((qtp, NQ), f32)
    qnum = qidx // LQ
    wind[np.arange(kq), qnum] = 1.0 / qsum[qnum]

    # ---- doc compaction ----
    cnt = (dmf > 0).sum(axis=1)
    # every doc keeps >=1 zero slot for the masked-score baseline;
    # ld=180 guarantees 192 always suffices
    ldp = 128 if cnt.max() < 128 else 192
    dtp = NDC * ldp
    ntd = dtp // TW

    in_maps = []
    for c in range(NCORES):
        dh_c = np.zeros((dtp, H), f32)
        dm_c = np.zeros(dtp, f32)
        for i in range(NDC):
            d = c * NDC + i
            idx = np.nonzero(dmf[d] > 0)[0]
            n = len(idx)
            dh_c[i * ldp:i * ldp + n] = dh2[d * LD + idx]
            dm_c[i * ldp:i * ldp + n] = 1.0
        in_maps.append({
            "dht": np.ascontiguousarray(dh_c.T),
            "qht": qht,
            "w1": w1,
            "w2": w2,
            "b1c": b1c,
            "b2c": b2c,
            "dmr": dm_c.reshape(ntd, TW),
            "id4": np.eye(4, dtype=f32),
            "wind": wind,
        })
    return in_maps, qtp, ldp


def kernel(query_hidden, doc_hidden, query_mask, doc_punct_mask,
           W1, b1, W2, b2):
    in_maps, qtp, ldp = _prep_inputs(query_hidden, doc_hidden, query_mask,
                                     doc_punct_mask, W1, b1, W2, b2)
    nc = _get_module(qtp, ldp)
    res = run_bass_kernel_spmd(nc, in_maps, list(range(NCORES)))
    _CACHE["last_results"] = res
    outs = [np.asarray(res.results[c]["out"]) for c in range(NCORES)]
    return np.concatenate(outs, axis=1).astype(np.float32)

